# revision 61
# baseline (speedup 1.0000x reference)
"""Trainium2 Bass kernel for nn_Event_Critic_Net (dual-branch GAT critic).

Math: the reference reads the GAT output only at the LAST node of each
graph (graphs are 32 contiguous nodes), so only edges (n -> last(g))
contribute.  For those the softmax-weighted aggregation commutes with
the projection W:

    out_g = sigmoid( (sum_n alpha[n] x[n,:]) @ W + bias )
    alpha[n] = cnt[n] e^{z[n]} / (sum_n cnt[n] e^{z[n]} + 1e-16)
    z[n] = leaky_relu(x[n].w_src + x[last(g)].w_dst),  w_* = W @ att_*

Only ~7 of 32 nodes per graph have cnt>0, so the host compacts
contributors into K slots per graph with ADAPTIVE K: per core the 512
graphs are sorted by descending max-branch contributor count and split
into regions K=16 (8 graphs/tile), K=12 (10/tile), K=8 (16/tile);
graphs with >16 contributors spill into overflow tiles aggregated in a
separate PSUM tile and merged in.  x is shipped once, node-major,
pre-scaled by w_src so a_src is a plain row-sum (DVE tensor_reduce);
the projection uses W' = W / w_src to undo the scaling.  Aggregation
runs on the PE with 64-column stationary x tiles and the per-slot
softmax weights M as the narrow moving operand.  Graphs are
data-parallel across the 8 cores.
"""

import numpy as np
from contextlib import ExitStack

NC = 8
N = 131072
G = 4096
NPG = 32
S = 64
H = 128
GPC = G // NC          # 512 graphs per core
NEG = 0.2
NWARM = 30             # PE clock warm-up matmuls

_CACHE = {}


def _tile_table(regions):
    """Per home tile: (col0, G_t, region_id). Cols == graph ranks."""
    tiles = []
    rank0 = 0
    for rid, (K, Gr, nt, ga) in enumerate(regions):
        for i in range(nt):
            c0 = rank0 + Gr * i
            gt = min(Gr, ga - Gr * i)
            tiles.append((c0, gt, rid))
        rank0 += ga
    return tiles


def _build_module(regions, OVU, OVD):
    import concourse.tile as tile
    from concourse import bacc, mybir
    from concourse.alu_op_type import AluOpType as Alu

    f32 = mybir.dt.float32
    bf16 = mybir.dt.bfloat16
    Act = mybir.ActivationFunctionType
    AxX = mybir.AxisListType.X

    THm = sum(r[2] for r in regions)          # home tiles
    TU = THm + OVU
    TD = THm + OVD
    tiles_tab = _tile_table(regions)
    rank0s = np.cumsum([0] + [r[3] for r in regions]).tolist()
    t0s = np.cumsum([0] + [r[2] for r in regions]).tolist()

    nc = bacc.Bacc("TRN2", target_bir_lowering=False, debug=False,
                   num_devices=NC)

    FW = 100 + TU + TD
    BW = 768
    XLW = max(r[1] for r in regions) * S
    XLR = max(32 * i + r[2] for i, r in enumerate(regions))
    dram = {
        "u_xn": nc.dram_tensor("u_xn", [128, (TU + 1) * S], bf16,
                               kind="ExternalInput"),
        "d_xn": nc.dram_tensor("d_xn", [128, (TD + 1) * S], bf16,
                               kind="ExternalInput"),
        "u_xl": nc.dram_tensor("u_xl", [XLR, XLW], bf16,
                               kind="ExternalInput"),
        "d_xl": nc.dram_tensor("d_xl", [XLR, XLW], bf16,
                               kind="ExternalInput"),
        "cstF": nc.dram_tensor("cstF", [128, FW], f32,
                               kind="ExternalInput"),
        "cstB": nc.dram_tensor("cstB", [128, BW], bf16,
                               kind="ExternalInput"),
    }
    out_dram = nc.dram_tensor("out", [1, GPC], f32, kind="ExternalOutput")

    def chunk_plan(T):
        s0, s3 = 8, 4
        rest = T - s0 - s3
        s1 = rest // 2
        sizes = [s0, s1, rest - s1, s3]
        out = []
        t0 = 0
        for sz in sizes:
            out.append((t0, sz))
            t0 += sz
        return out

    CH = {"u": chunk_plan(TU), "d": chunk_plan(TD)}
    OV_ = {"u": OVU, "d": OVD}

    with tile.TileContext(nc) as tc, ExitStack() as ctx:
        const = ctx.enter_context(tc.tile_pool(name="const", bufs=1))
        xp = ctx.enter_context(tc.tile_pool(name="xp", bufs=1))
        wk = ctx.enter_context(tc.tile_pool(name="wk", bufs=1))
        pmix = ctx.enter_context(tc.tile_pool(name="pmix", bufs=2,
                                              space="PSUM"))
        pdn = ctx.enter_context(tc.tile_pool(name="pdn", bufs=2,
                                             space="PSUM"))
        py = ctx.enter_context(tc.tile_pool(name="py", bufs=2,
                                            space="PSUM"))
        pbig = ctx.enter_context(tc.tile_pool(name="pbig", bufs=2,
                                              space="PSUM"))

        # ---- warm-up + constants ----
        wsrc = const.tile([64, 72], bf16, tag="wsrc")
        nc.vector.memset(wsrc[:], 1.0)

        cstB = const.tile([128, BW], bf16, tag="cstB")
        nc.sync.dma_start(cstB[:], dram["cstB"].ap())
        cstF = const.tile([128, FW], f32, tag="cstF")
        nc.scalar.dma_start(cstF[:], dram["cstF"].ap())
        xl = {}
        for p, eng in (("u", nc.sync), ("d", nc.scalar)):
            t = const.tile([XLR, XLW], bf16, tag=f"xl{p}", name=f"xl{p}")
            eng.dma_start(t[:], dram[f"{p}_xl"].ap())
            xl[p] = t

        for wi in range(NWARM):
            w_ps = pmix.tile([128, 64], f32, tag="mix", name=f"warm{wi}")
            nc.tensor.matmul(w_ps[0:64, 0:8], wsrc[:, 0:64],
                             wsrc[:, 64:72], start=True, stop=True)

        # const views
        pbias = {"u": cstF[:, 0:1], "d": cstF[:, 1:2]}
        eye = cstF[0:96, 4:100]
        CT = {"u": cstF[:, 100:100 + TU],
              "d": cstF[:, 100 + TU:100 + TU + TD]}
        ones_col = cstB[:, 136:137]
        ones64 = cstB[0:1, 137:201]
        Wp = {"u": cstB[0:64, 201:329], "d": cstB[0:64, 329:457]}
        mlpW = cstB[:, 457:458]
        # Q [G, 128] and mask [128, G] keyed by graphs-per-tile
        Qg = {8: cstB[0:8, 0:128], 10: cstB[0:10, 458:586],
              16: cstB[0:16, 586:714]}
        Mg = {8: cstB[:, 128:136], 10: cstB[:, 714:724],
              16: cstB[:, 724:740]}
        Qr = [Qg[r[1]] for r in regions]
        Mk = [Mg[r[1]] for r in regions]

        # ---- big input DMAs (chunked, branch per queue) ----
        xt = {"u": [], "d": []}
        for p in ("u", "d"):
            for ci, (t0, nt) in enumerate(CH[p]):
                t = xp.tile([128, (nt + 1) * S], bf16, tag=f"xn{p}{ci}",
                            name=f"xn{p}{ci}")
                xt[p].append(t)
        for ci in range(len(CH["u"])):
            for p, eng in (("u", nc.sync), ("d", nc.scalar)):
                t0, nt = CH[p][ci]
                eng.dma_start(
                    xt[p][ci][:],
                    dram[f"{p}_xn"].ap()[:, t0 * S:(t0 + nt + 1) * S])

        # ---- a_dst paths (per branch x region) ----
        AD = {"u": [], "d": []}
        for p in ("u", "d"):
            for rid, (K, Gr, nt, ga) in enumerate(regions):
                rb = 32 * rid
                ad = wk.tile([96, Gr], f32, tag=f"ad{p}{rid}",
                             name=f"ad{p}{rid}")
                nc.vector.tensor_reduce(
                    ad[rb:rb + nt, :],
                    xl[p][rb:rb + nt, 0:Gr * S]
                        .rearrange("p (j s) -> p j s", s=S),
                    axis=AxX, op=Alu.add)
                tp = pmix.tile([128, 64], f32, tag="mix",
                               name=f"tp{p}{rid}")
                nc.tensor.transpose(tp[0:Gr, 0:nt], ad[rb:rb + nt, :],
                                    eye[rb:rb + nt, rb:rb + nt])
                adT = wk.tile([Gr, nt], bf16, tag=f"adT{p}{rid}",
                              name=f"adT{p}{rid}")
                nc.scalar.activation(adT[:], tp[0:Gr, 0:nt], Act.Copy)
                ad_ps = pmix.tile([128, 64], f32, tag="mix",
                                  name=f"adps{p}{rid}")
                nc.tensor.matmul(ad_ps[:, 0:nt], Qr[rid], adT[:],
                                 start=True, stop=True)
                a = wk.tile([128, nt], f32, tag=f"AD{p}{rid}",
                            name=f"AD{p}{rid}")
                nc.scalar.activation(a[:], ad_ps[:, 0:nt], Act.Copy)
                AD[p].append(a)

        # ---- per-branch state ----
        st = {}
        for p in ("u", "d"):
            T = THm + OV_[p]
            st[p] = {
                "AS": wk.tile([128, T], f32, tag=f"AS{p}", name=f"AS{p}"),
                "z": wk.tile([128, T], f32, tag=f"z{p}", name=f"z{p}"),
                "e": wk.tile([128, T], f32, tag=f"e{p}", name=f"e{p}"),
                "EX": wk.tile([128, T], f32, tag=f"EX{p}", name=f"EX{p}"),
                "P": wk.tile([128, T], f32, tag=f"P{p}", name=f"P{p}"),
                "M": wk.tile([128, GPC + 8 * OV_[p]], bf16, tag=f"M{p}",
                             name=f"M{p}"),
                "ynT": py.tile([128, GPC], f32, tag="ynT",
                               name=f"ynT{p}"),
                "dn": pdn.tile([1, GPC], f32, tag="dn", name=f"dn{p}"),
                "ov": pmix.tile([128, 64], f32, tag="mix", name=f"ov{p}"),
                "dnb": wk.tile([1, GPC], bf16, tag=f"dnb{p}",
                               name=f"dnb{p}"),
                "rbc": pbig.tile([64, GPC], f32, tag="big",
                                 name=f"rbc{p}"),
                "rinv": wk.tile([64, GPC], f32, tag=f"rinv{p}",
                                name=f"rinv{p}"),
                "ynrm": wk.tile([64, GPC], bf16, tag=f"ynrm{p}",
                                name=f"ynrm{p}"),
                "hT": pbig.tile([128, GPC], f32, tag="big",
                                name=f"hT{p}"),
                "sg": wk.tile([128, GPC], bf16, tag=f"sg{p}",
                              name=f"sg{p}"),
            }

        def col_of(tid):
            if tid < THm:
                return tiles_tab[tid][0]
            return GPC + 8 * (tid - THm)

        def segments(t0, nt):
            """Split home-tile range [t0, t0+nt) into per-region,
            per-uniform-G segments: (tile0, ntiles, rid, Gseg)."""
            out = []
            a = t0
            end = min(t0 + nt, THm)
            while a < end:
                rid = tiles_tab[a][2]
                K, Gr, ntr, ga = regions[rid]
                b = min(end, t0s[rid] + ntr)
                # peel off trailing partial tile (G_t < Gr)
                full_end = b
                if tiles_tab[b - 1][1] != Gr:
                    full_end = b - 1
                if full_end > a:
                    out.append((a, full_end - a, rid, Gr))
                for t in range(max(full_end, a), b):
                    out.append((t, 1, rid, tiles_tab[t][1]))
                a = b
            return out

        def reduce_chunk(p, ci):
            t0, nt = CH[p][ci]
            s = st[p]
            # stage 1: pairwise feature-half add in bf16 (2x packed DVE
            # mode), stage 2: half-volume grouped reduce to f32
            h1 = wk.tile([128, nt * 32], bf16, tag=f"h1{p}{ci}",
                         name=f"h1{p}{ci}")
            x3 = xt[p][ci][:, 0:nt * S].rearrange("p (t s) -> p t s",
                                                  s=S)
            nc.vector.tensor_tensor(
                h1[:].rearrange("p (t s) -> p t s", s=32),
                x3[:, :, 0:32], x3[:, :, 32:64], op=Alu.add)
            nc.vector.tensor_reduce(
                s["AS"][:, t0:t0 + nt],
                h1[:].rearrange("p (t s) -> p t s", s=32),
                axis=AxX, op=Alu.add)

        def mchain_chunk(p, ci):
            t0, nt = CH[p][ci]
            s = st[p]
            for (a, n_, rid, _) in segments(t0, nt):
                nc.gpsimd.tensor_tensor(
                    s["z"][:, a:a + n_], s["AS"][:, a:a + n_],
                    AD[p][rid][:, a - t0s[rid]:a - t0s[rid] + n_],
                    op=Alu.add)
            if t0 + nt > THm:           # overflow tiles: region-0 AD
                o0 = max(t0, THm)
                nb = t0 + nt - o0
                nc.gpsimd.tensor_tensor(
                    s["z"][:, o0:o0 + nb], s["AS"][:, o0:o0 + nb],
                    AD[p][0][:, o0 - THm:o0 - THm + nb], op=Alu.add)
            sl = slice(t0, t0 + nt)
            nc.vector.scalar_tensor_tensor(
                s["e"][:, sl], s["z"][:, sl], NEG, s["z"][:, sl],
                op0=Alu.mult, op1=Alu.max)
            nc.scalar.activation(s["EX"][:, sl], s["e"][:, sl], Act.Exp)
            nc.gpsimd.tensor_tensor(
                s["P"][:, sl], s["EX"][:, sl], CT[p][:, sl], op=Alu.mult)

        def mbuild_chunk(p, ci):
            t0, nt = CH[p][ci]
            s = st[p]
            for (a, n_, rid, Gseg) in segments(t0, nt):
                c0 = col_of(a)
                nc.vector.tensor_tensor(
                    s["M"][:, c0:c0 + n_ * Gseg]
                        .rearrange("p (t j) -> p t j", j=Gseg),
                    s["P"][:, a:a + n_].rearrange("p (t o) -> p t o", o=1)
                        .to_broadcast((128, n_, Gseg)),
                    Mk[rid][:, 0:Gseg].rearrange("p (o j) -> p o j", o=1)
                        .to_broadcast((128, n_, Gseg)),
                    op=Alu.mult)
            if t0 + nt > THm:
                o0 = max(t0, THm)
                nb = t0 + nt - o0
                c0 = col_of(o0)
                nc.vector.tensor_tensor(
                    s["M"][:, c0:c0 + nb * 8]
                        .rearrange("p (t j) -> p t j", j=8),
                    s["P"][:, o0:o0 + nb].rearrange("p (t o) -> p t o", o=1)
                        .to_broadcast((128, nb, 8)),
                    Mk[0].rearrange("p (o j) -> p o j", o=1)
                        .to_broadcast((128, nb, 8)),
                    op=Alu.mult)

        def agg_chunk(p, ci):
            t0, nt = CH[p][ci]
            s = st[p]
            OV = OV_[p]
            x = xt[p][ci]
            for i in range(nt):
                tid = t0 + i
                c0 = col_of(tid)
                if tid < THm:
                    gt = tiles_tab[tid][1]
                    nc.tensor.matmul(
                        s["ynT"][:, c0:c0 + gt],
                        x[:, S * i:S * i + 128],
                        s["M"][:, c0:c0 + gt],
                        start=True, stop=True)
                else:
                    b = tid - THm
                    nc.tensor.matmul(
                        s["ov"][:, 8 * b:8 * b + 8],
                        x[:, S * i:S * i + 128],
                        s["M"][:, c0:c0 + 8],
                        start=True, stop=True)
            h0, h1 = t0, min(t0 + nt, THm)
            if h1 > h0:
                nc.tensor.matmul(
                    s["dn"][:, col_of(h0):col_of(h1 - 1) +
                            tiles_tab[h1 - 1][1]],
                    ones_col,
                    s["M"][:, col_of(h0):col_of(h1 - 1) +
                           tiles_tab[h1 - 1][1]],
                    start=True, stop=True)
            if t0 + nt > THm:
                o0 = max(t0, THm)
                nb = t0 + nt - o0
                nc.tensor.matmul(
                    s["ov"][0:1, 8 * OV:8 * OV + 8 * nb], ones_col,
                    s["M"][:, col_of(o0):col_of(o0) + 8 * nb],
                    start=True, stop=True)

        HS = GPC // 2

        def tail_ov(p):
            s = st[p]
            OV = OV_[p]
            ovsb = wk.tile([64, 16 * OV], f32, tag=f"ovsb{p}",
                           name=f"ovsb{p}")
            nc.scalar.activation(ovsb[:], s["ov"][0:64, 0:16 * OV],
                                 Act.Copy)
            nc.vector.tensor_tensor(
                s["ynT"][0:64, 0:8 * OV], s["ynT"][0:64, 0:8 * OV],
                ovsb[:, 0:8 * OV], op=Alu.add)
            nc.vector.tensor_tensor(
                s["dn"][:, 0:8 * OV], s["dn"][:, 0:8 * OV],
                ovsb[0:1, 8 * OV:16 * OV], op=Alu.add)

        def tail_a(p, h):
            s = st[p]
            c = slice(h * HS, (h + 1) * HS)
            nc.scalar.activation(s["dnb"][:, c], s["dn"][:, c], Act.Copy,
                                 bias=1e-16)
            nc.tensor.matmul(s["rbc"][:, c], ones64, s["dnb"][:, c],
                             start=True, stop=True)
            nc.vector.reciprocal_approx_fast(s["rinv"][:, c],
                                             s["rbc"][:, c])
            nc.vector.tensor_tensor(s["ynrm"][:, c], s["ynT"][0:64, c],
                                    s["rinv"][:, c], op=Alu.mult)

        def tail_b(p, h):
            s = st[p]
            c = slice(h * HS, (h + 1) * HS)
            nc.tensor.matmul(s["hT"][:, c], Wp[p], s["ynrm"][:, c],
                             start=True, stop=True)
            nc.scalar.activation(s["sg"][:, c], s["hT"][:, c], Act.Sigmoid,
                                 bias=pbias[p])

        # ---- schedule ----
        for ci in range(len(CH["u"])):
            for p in ("u", "d"):
                reduce_chunk(p, ci)
                mchain_chunk(p, ci)
                mbuild_chunk(p, ci)
                agg_chunk(p, ci)
        for p in ("u", "d"):
            tail_ov(p)
            for h in (1, 0):
                tail_a(p, h)
                tail_b(p, h)

        # ---- head ----
        prod = wk.tile([128, GPC], bf16, tag="prod")
        o_ps = pdn.tile([1, GPC], f32, tag="dn", name="o_ps")
        o_sb = wk.tile([1, GPC], f32, tag="o_sb")
        for h in (1, 0):
            c = slice(h * HS, (h + 1) * HS)
            nc.vector.tensor_tensor(prod[:, c], st["u"]["sg"][:, c],
                                    st["d"]["sg"][:, c], op=Alu.mult)
            nc.tensor.matmul(o_ps[:, c], mlpW, prod[:, c], start=True,
                             stop=True)
            nc.scalar.activation(o_sb[:, c], o_ps[:, c], Act.Copy)
        nc.sync.dma_start(out_dram.ap(), o_sb[:])

    nc.compile()
    return nc


def _get_module(regions, OVU, OVD):
    key = ("nc", regions, OVU, OVD)
    if key not in _CACHE:
        _CACHE[key] = _build_module(regions, OVU, OVD)
    return _CACHE[key]


# ---------------- host-side prep ----------------

def _branch_struct(ei):
    src = np.asarray(ei[0]).astype(np.int64)
    dst = np.asarray(ei[1]).astype(np.int64)
    valid = (dst % NPG) == (NPG - 1)
    cnt = np.bincount(src[valid], minlength=N).astype(np.float32)
    contrib = (cnt > 0).reshape(G, NPG).sum(1)
    return cnt, contrib


def _clamp_w(w):
    w = np.asarray(w, np.float64).copy()
    tiny = np.abs(w) < 1e-4
    w[tiny] = np.where(w[tiny] < 0, -1e-4, 1e-4)
    return w


def _plan_regions(orders, con_u, con_d):
    import os
    mode = os.environ.get("REGIONS", "1")
    if mode == "2":
        ngt8 = 0
        for order in orders:
            mx = np.maximum(con_u[order], con_d[order])
            ngt8 = max(ngt8, int((mx > 8).sum()))
        g16 = max(8, -(-ngt8 // 8) * 8)
        g8 = GPC - g16
        regions = ((16, 8, g16 // 8, g16), (8, 16, -(-g8 // 16), g8))
        assert regions[0][2] <= 32 and regions[1][2] <= 32
        return tuple(r for r in regions if r[2] > 0)
    return ((16, 8, GPC // 8, GPC),)


def _overflow_tiles(orders, cnt):
    nb = 0
    for order in orders:
        counts = np.array([(cnt[g * NPG:(g + 1) * NPG] > 0).sum()
                           for g in order])
        assert counts.max() <= 32, "needs level-2 overflow support"
        ranks = np.nonzero(counts > 16)[0]
        if len(ranks):
            nb = max(nb, int(ranks.max() // 8 + 1))
    return nb


def _pack_branch(x, cnt, orders, w_src, w_dst, regions, OV):
    import ml_dtypes
    bf = ml_dtypes.bfloat16
    x = np.asarray(x, np.float32)
    wc = _clamp_w(w_src).astype(np.float32)
    THm = sum(r[2] for r in regions)
    T = THm + OV
    rank0s = np.cumsum([0] + [r[3] for r in regions])
    t0s = np.cumsum([0] + [r[2] for r in regions])
    per_core = []
    for c in range(NC):
        order = orders[c]
        XN = np.zeros((128, (T + 1) * S), np.float32)
        CTm = np.zeros((128, T), np.float32)
        XLW = max(r[1] for r in regions) * S
        XLR = max(32 * i + r[2] for i, r in enumerate(regions))
        XL = np.zeros((XLR, XLW), np.float32)
        for r, g in enumerate(order):
            rid = int(np.searchsorted(rank0s, r, side="right") - 1)
            K, Gr, ntr, ga = regions[rid]
            rr = r - rank0s[rid]
            t, j = rr // Gr, rr % Gr
            tid = t0s[rid] + t
            xlrow = 32 * rid + t
            nodes = np.nonzero(cnt[g * NPG:(g + 1) * NPG] > 0)[0] + g * NPG
            assert len(nodes) <= K or rid == 0
            XL[xlrow, j * S:(j + 1) * S] = x[(g + 1) * NPG - 1] * w_dst
            seg = nodes[:K]
            p0 = K * j
            XN[p0:p0 + len(seg), tid * S:tid * S + S] = x[seg] * wc
            CTm[p0:p0 + len(seg), tid] = cnt[seg]
            if len(nodes) > K:          # overflow (region 0 only)
                assert rid == 0
                seg2 = nodes[K:]
                otid = THm + t
                p0 = 16 * j
                XN[p0:p0 + len(seg2), otid * S:otid * S + S] = x[seg2] * wc
                CTm[p0:p0 + len(seg2), otid] = cnt[seg2]
        per_core.append({"XN": XN.astype(bf), "CT": CTm,
                         "XL": XL.astype(bf)})
    return per_core, wc


def _build_in_maps(inputs):
    import ml_dtypes
    bf = ml_dtypes.bfloat16

    cnt_u, con_u = _branch_struct(inputs["up_edge_index"])
    cnt_d, con_d = _branch_struct(inputs["down_edge_index"])
    orders = []
    for c in range(NC):
        g0 = c * GPC
        mx = np.maximum(con_u[g0:g0 + GPC], con_d[g0:g0 + GPC])
        orders.append(np.argsort(-mx, kind="stable") + g0)
    regions = _plan_regions(orders, con_u, con_d)
    OVU = max(1, _overflow_tiles(orders, cnt_u))
    OVD = max(1, _overflow_tiles(orders, cnt_d))
    assert OVU <= regions[0][2] and OVD <= regions[0][2]
    THm = sum(r[2] for r in regions)
    TU, TD = THm + OVU, THm + OVD

    pcs = {}
    shr = {}
    for pref, p, cnt, OV in (("up", "u", cnt_u, OVU),
                             ("down", "d", cnt_d, OVD)):
        W = np.asarray(inputs[f"{pref}_W"], np.float32)
        w_src = W @ np.asarray(inputs[f"{pref}_att_src"], np.float32)
        w_dst = W @ np.asarray(inputs[f"{pref}_att_dst"], np.float32)
        pcs[p], wc = _pack_branch(inputs[f"{pref}_x"], cnt, orders,
                                  w_src, w_dst, regions, OV)
        shr[p] = {
            "Wp": (W / wc[:, None]).astype(np.float32),
            "bias": np.asarray(inputs[f"{pref}_bias"], np.float32),
        }

    FW = 100 + TU + TD
    cstF = np.zeros((128, FW), np.float32)
    cstF[:, 0] = shr["u"]["bias"]
    cstF[:, 1] = shr["d"]["bias"]
    cstF[0, 2] = 1e-16
    cstF[0:96, 4:100] = np.eye(96, dtype=np.float32)

    cstB = np.zeros((128, 768), np.float32)
    pp = np.arange(128)
    # region masks/Q: K=16 -> G=8 @ (0:128, 128:136); K=12 -> G=10;
    # K=8 -> G=16
    q16 = np.zeros((8, 128), np.float32)
    q16[pp // 16, pp] = 1.0
    cstB[0:8, 0:128] = q16
    cstB[:, 128:136] = q16.T
    q12 = np.zeros((10, 128), np.float32)
    sel = pp < 120
    q12[pp[sel] // 12, pp[sel]] = 1.0
    cstB[0:10, 458:586] = q12
    cstB[:, 714:724] = q12.T
    q8 = np.zeros((16, 128), np.float32)
    q8[pp // 8, pp] = 1.0
    cstB[0:16, 586:714] = q8
    cstB[:, 724:740] = q8.T
    cstB[:, 136] = 1.0
    cstB[0, 137:201] = 1.0
    cstB[0:64, 201:329] = shr["u"]["Wp"]
    cstB[0:64, 329:457] = shr["d"]["Wp"]
    cstB[:, 457] = np.asarray(inputs["mlp_W"], np.float32).reshape(H)

    in_maps = []
    for c in range(NC):
        cf = cstF.copy()
        cf[:, 100:100 + TU] = pcs["u"][c]["CT"]
        cf[:, 100 + TU:100 + TU + TD] = pcs["d"][c]["CT"]
        m = {
            "cstF": cf,
            "cstB": cstB.astype(bf),
            "u_xn": pcs["u"][c]["XN"],
            "d_xn": pcs["d"][c]["XN"],
            "u_xl": pcs["u"][c]["XL"],
            "d_xl": pcs["d"][c]["XL"],
        }
        in_maps.append(m)
    meta = {"orders": orders, "OVU": OVU, "OVD": OVD, "regions": regions,
            "mlp_b": float(np.asarray(inputs["mlp_b"]).reshape(-1)[0])}
    return in_maps, meta


def assemble(results, meta):
    out = np.zeros((G, 1), np.float32)
    for c in range(NC):
        o = np.asarray(results[c]["out"], np.float32).reshape(GPC)
        out[meta["orders"][c], 0] = o + meta["mlp_b"]
    return out


def kernel(**inputs):
    from concourse.bass_utils import run_bass_kernel_spmd

    in_maps, meta = _build_in_maps(inputs)
    nc = _get_module(meta["regions"], meta["OVU"], meta["OVD"])
    res = run_bass_kernel_spmd(nc, in_maps, core_ids=list(range(NC)))
    return assemble(res.results, meta)


# revision 62
# speedup vs baseline: 1.0050x; 1.0050x over previous
"""Trainium2 Bass kernel for nn_Event_Critic_Net (dual-branch GAT critic).

Math: the reference reads the GAT output only at the LAST node of each
graph (graphs are 32 contiguous nodes), so only edges (n -> last(g))
contribute.  For those the softmax-weighted aggregation commutes with
the projection W:

    out_g = sigmoid( (sum_n alpha[n] x[n,:]) @ W + bias )
    alpha[n] = cnt[n] e^{z[n]} / (sum_n cnt[n] e^{z[n]} + 1e-16)
    z[n] = leaky_relu(x[n].w_src + x[last(g)].w_dst),  w_* = W @ att_*

Only ~7 of 32 nodes per graph have cnt>0, so the host compacts
contributors into K slots per graph with ADAPTIVE K: per core the 512
graphs are sorted by descending max-branch contributor count and split
into regions K=16 (8 graphs/tile), K=12 (10/tile), K=8 (16/tile);
graphs with >16 contributors spill into overflow tiles aggregated in a
separate PSUM tile and merged in.  x is shipped once, node-major,
pre-scaled by w_src so a_src is a plain row-sum (DVE tensor_reduce);
the projection uses W' = W / w_src to undo the scaling.  Aggregation
runs on the PE with 64-column stationary x tiles and the per-slot
softmax weights M as the narrow moving operand.  Graphs are
data-parallel across the 8 cores.
"""

import numpy as np
from contextlib import ExitStack

NC = 8
N = 131072
G = 4096
NPG = 32
S = 64
H = 128
GPC = G // NC          # 512 graphs per core
NEG = 0.2
NWARM = 30             # PE clock warm-up matmuls

_CACHE = {}


def _tile_table(regions):
    """Per home tile: (col0, G_t, region_id). Cols == graph ranks."""
    tiles = []
    rank0 = 0
    for rid, (K, Gr, nt, ga) in enumerate(regions):
        for i in range(nt):
            c0 = rank0 + Gr * i
            gt = min(Gr, ga - Gr * i)
            tiles.append((c0, gt, rid))
        rank0 += ga
    return tiles


def _build_module(regions, OVU, OVD):
    import concourse.tile as tile
    from concourse import bacc, mybir
    from concourse.alu_op_type import AluOpType as Alu

    f32 = mybir.dt.float32
    bf16 = mybir.dt.bfloat16
    Act = mybir.ActivationFunctionType
    AxX = mybir.AxisListType.X

    THm = sum(r[2] for r in regions)          # home tiles
    TU = THm + OVU
    TD = THm + OVD
    tiles_tab = _tile_table(regions)
    rank0s = np.cumsum([0] + [r[3] for r in regions]).tolist()
    t0s = np.cumsum([0] + [r[2] for r in regions]).tolist()

    nc = bacc.Bacc("TRN2", target_bir_lowering=False, debug=False,
                   num_devices=NC)

    FW = 100 + TU + TD
    BW = 768
    XLW = max(r[1] for r in regions) * S
    XLR = max(32 * i + r[2] for i, r in enumerate(regions))
    dram = {
        "u_xn": nc.dram_tensor("u_xn", [128, (TU + 1) * S], bf16,
                               kind="ExternalInput"),
        "d_xn": nc.dram_tensor("d_xn", [128, (TD + 1) * S], bf16,
                               kind="ExternalInput"),
        "u_xl": nc.dram_tensor("u_xl", [XLR, XLW], bf16,
                               kind="ExternalInput"),
        "d_xl": nc.dram_tensor("d_xl", [XLR, XLW], bf16,
                               kind="ExternalInput"),
        "cstF": nc.dram_tensor("cstF", [128, FW], f32,
                               kind="ExternalInput"),
        "cstB": nc.dram_tensor("cstB", [128, BW], bf16,
                               kind="ExternalInput"),
    }
    out_dram = nc.dram_tensor("out", [1, GPC], f32, kind="ExternalOutput")

    def chunk_plan(T):
        s0, s3 = 8, 4
        rest = T - s0 - s3
        s1 = rest // 2
        sizes = [s0, s1, rest - s1, s3]
        out = []
        t0 = 0
        for sz in sizes:
            out.append((t0, sz))
            t0 += sz
        return out

    CH = {"u": chunk_plan(TU), "d": chunk_plan(TD)}
    OV_ = {"u": OVU, "d": OVD}

    with tile.TileContext(nc) as tc, ExitStack() as ctx:
        const = ctx.enter_context(tc.tile_pool(name="const", bufs=1))
        xp = ctx.enter_context(tc.tile_pool(name="xp", bufs=1))
        wk = ctx.enter_context(tc.tile_pool(name="wk", bufs=1))
        pmix = ctx.enter_context(tc.tile_pool(name="pmix", bufs=2,
                                              space="PSUM"))
        pdn = ctx.enter_context(tc.tile_pool(name="pdn", bufs=2,
                                             space="PSUM"))
        py = ctx.enter_context(tc.tile_pool(name="py", bufs=2,
                                            space="PSUM"))
        pbig = ctx.enter_context(tc.tile_pool(name="pbig", bufs=2,
                                              space="PSUM"))

        # ---- warm-up + constants ----
        wsrc = const.tile([64, 72], bf16, tag="wsrc")
        nc.vector.memset(wsrc[:], 1.0)

        cstB = const.tile([128, BW], bf16, tag="cstB")
        nc.sync.dma_start(cstB[:], dram["cstB"].ap())
        cstF = const.tile([128, FW], f32, tag="cstF")
        nc.scalar.dma_start(cstF[:], dram["cstF"].ap())
        xl = {}
        for p, eng in (("u", nc.sync), ("d", nc.scalar)):
            t = const.tile([XLR, XLW], bf16, tag=f"xl{p}", name=f"xl{p}")
            eng.dma_start(t[:], dram[f"{p}_xl"].ap())
            xl[p] = t

        for wi in range(NWARM):
            w_ps = pmix.tile([128, 64], f32, tag="mix", name=f"warm{wi}")
            nc.tensor.matmul(w_ps[0:64, 0:8], wsrc[:, 0:64],
                             wsrc[:, 64:72], start=True, stop=True)

        # const views
        pbias = {"u": cstF[:, 0:1], "d": cstF[:, 1:2]}
        eye = cstF[0:96, 4:100]
        CT = {"u": cstF[:, 100:100 + TU],
              "d": cstF[:, 100 + TU:100 + TU + TD]}
        ones_col = cstB[:, 136:137]
        ones64 = cstB[0:1, 137:201]
        Wp = {"u": cstB[0:64, 201:329], "d": cstB[0:64, 329:457]}
        mlpW = cstB[:, 457:458]
        # Q [G, 128] and mask [128, G] keyed by graphs-per-tile
        Qg = {8: cstB[0:8, 0:128], 10: cstB[0:10, 458:586],
              16: cstB[0:16, 586:714]}
        Mg = {8: cstB[:, 128:136], 10: cstB[:, 714:724],
              16: cstB[:, 724:740]}
        Qr = [Qg[r[1]] for r in regions]
        Mk = [Mg[r[1]] for r in regions]

        # ---- big input DMAs (chunked, branch per queue) ----
        xt = {"u": [], "d": []}
        for p in ("u", "d"):
            for ci, (t0, nt) in enumerate(CH[p]):
                t = xp.tile([128, (nt + 1) * S], bf16, tag=f"xn{p}{ci}",
                            name=f"xn{p}{ci}")
                xt[p].append(t)
        for ci in range(len(CH["u"])):
            for p, eng in (("u", nc.sync), ("d", nc.scalar)):
                t0, nt = CH[p][ci]
                eng.dma_start(
                    xt[p][ci][:],
                    dram[f"{p}_xn"].ap()[:, t0 * S:(t0 + nt + 1) * S])

        # ---- a_dst paths (per branch x region) ----
        AD = {"u": [], "d": []}
        for p in ("u", "d"):
            for rid, (K, Gr, nt, ga) in enumerate(regions):
                rb = 32 * rid
                ad = wk.tile([96, Gr], f32, tag=f"ad{p}{rid}",
                             name=f"ad{p}{rid}")
                nc.vector.tensor_reduce(
                    ad[rb:rb + nt, :],
                    xl[p][rb:rb + nt, 0:Gr * S]
                        .rearrange("p (j s) -> p j s", s=S),
                    axis=AxX, op=Alu.add)
                tp = pmix.tile([128, 64], f32, tag="mix",
                               name=f"tp{p}{rid}")
                nc.tensor.transpose(tp[0:Gr, 0:nt], ad[rb:rb + nt, :],
                                    eye[rb:rb + nt, rb:rb + nt])
                adT = wk.tile([Gr, nt], bf16, tag=f"adT{p}{rid}",
                              name=f"adT{p}{rid}")
                nc.scalar.activation(adT[:], tp[0:Gr, 0:nt], Act.Copy)
                ad_ps = pmix.tile([128, 64], f32, tag="mix",
                                  name=f"adps{p}{rid}")
                nc.tensor.matmul(ad_ps[:, 0:nt], Qr[rid], adT[:],
                                 start=True, stop=True)
                a = wk.tile([128, nt], f32, tag=f"AD{p}{rid}",
                            name=f"AD{p}{rid}")
                nc.scalar.activation(a[:], ad_ps[:, 0:nt], Act.Copy)
                AD[p].append(a)

        # ---- per-branch state ----
        st = {}
        for p in ("u", "d"):
            T = THm + OV_[p]
            st[p] = {
                "AS": wk.tile([128, T], f32, tag=f"AS{p}", name=f"AS{p}"),
                "z": wk.tile([128, T], f32, tag=f"z{p}", name=f"z{p}"),
                "e": wk.tile([128, T], f32, tag=f"e{p}", name=f"e{p}"),
                "EX": wk.tile([128, T], f32, tag=f"EX{p}", name=f"EX{p}"),
                "P": wk.tile([128, T], f32, tag=f"P{p}", name=f"P{p}"),
                "M": wk.tile([128, GPC + 8 * OV_[p]], bf16, tag=f"M{p}",
                             name=f"M{p}"),
                "ynT": py.tile([128, GPC], f32, tag="ynT",
                               name=f"ynT{p}"),
                "dn": pdn.tile([1, GPC], f32, tag="dn", name=f"dn{p}"),
                "ov": pmix.tile([128, 64], f32, tag="mix", name=f"ov{p}"),
                "dnb": wk.tile([1, GPC], bf16, tag=f"dnb{p}",
                               name=f"dnb{p}"),
                "rbc": pbig.tile([64, GPC], f32, tag="big",
                                 name=f"rbc{p}"),
                "rinv": wk.tile([64, GPC], f32, tag=f"rinv{p}",
                                name=f"rinv{p}"),
                "ynrm": wk.tile([64, GPC], bf16, tag=f"ynrm{p}",
                                name=f"ynrm{p}"),
                "hT": pbig.tile([128, GPC], f32, tag="big",
                                name=f"hT{p}"),
                "sg": wk.tile([128, GPC], bf16, tag=f"sg{p}",
                              name=f"sg{p}"),
            }

        def col_of(tid):
            if tid < THm:
                return tiles_tab[tid][0]
            return GPC + 8 * (tid - THm)

        def segments(t0, nt):
            """Split home-tile range [t0, t0+nt) into per-region,
            per-uniform-G segments: (tile0, ntiles, rid, Gseg)."""
            out = []
            a = t0
            end = min(t0 + nt, THm)
            while a < end:
                rid = tiles_tab[a][2]
                K, Gr, ntr, ga = regions[rid]
                b = min(end, t0s[rid] + ntr)
                # peel off trailing partial tile (G_t < Gr)
                full_end = b
                if tiles_tab[b - 1][1] != Gr:
                    full_end = b - 1
                if full_end > a:
                    out.append((a, full_end - a, rid, Gr))
                for t in range(max(full_end, a), b):
                    out.append((t, 1, rid, tiles_tab[t][1]))
                a = b
            return out

        def reduce_chunk(p, ci):
            t0, nt = CH[p][ci]
            s = st[p]
            # stage 1: pairwise feature-half add in bf16 (2x packed DVE
            # mode), stage 2: half-volume grouped reduce to f32
            h1 = wk.tile([128, nt * 32], bf16, tag=f"h1{p}{ci}",
                         name=f"h1{p}{ci}")
            x3 = xt[p][ci][:, 0:nt * S].rearrange("p (t s) -> p t s",
                                                  s=S)
            nc.vector.tensor_tensor(
                h1[:].rearrange("p (t s) -> p t s", s=32),
                x3[:, :, 0:32], x3[:, :, 32:64], op=Alu.add)
            nc.vector.tensor_reduce(
                s["AS"][:, t0:t0 + nt],
                h1[:].rearrange("p (t s) -> p t s", s=32),
                axis=AxX, op=Alu.add)

        def mchain_chunk(p, ci):
            t0, nt = CH[p][ci]
            s = st[p]
            for (a, n_, rid, _) in segments(t0, nt):
                nc.vector.tensor_tensor(
                    s["z"][:, a:a + n_], s["AS"][:, a:a + n_],
                    AD[p][rid][:, a - t0s[rid]:a - t0s[rid] + n_],
                    op=Alu.add)
            if t0 + nt > THm:           # overflow tiles: region-0 AD
                o0 = max(t0, THm)
                nb = t0 + nt - o0
                nc.vector.tensor_tensor(
                    s["z"][:, o0:o0 + nb], s["AS"][:, o0:o0 + nb],
                    AD[p][0][:, o0 - THm:o0 - THm + nb], op=Alu.add)
            sl = slice(t0, t0 + nt)
            nc.vector.scalar_tensor_tensor(
                s["e"][:, sl], s["z"][:, sl], NEG, s["z"][:, sl],
                op0=Alu.mult, op1=Alu.max)
            nc.scalar.activation(s["EX"][:, sl], s["e"][:, sl], Act.Exp)
            nc.vector.tensor_tensor(
                s["P"][:, sl], s["EX"][:, sl], CT[p][:, sl], op=Alu.mult)

        def mbuild_chunk(p, ci):
            t0, nt = CH[p][ci]
            s = st[p]
            for (a, n_, rid, Gseg) in segments(t0, nt):
                c0 = col_of(a)
                nc.vector.tensor_tensor(
                    s["M"][:, c0:c0 + n_ * Gseg]
                        .rearrange("p (t j) -> p t j", j=Gseg),
                    s["P"][:, a:a + n_].rearrange("p (t o) -> p t o", o=1)
                        .to_broadcast((128, n_, Gseg)),
                    Mk[rid][:, 0:Gseg].rearrange("p (o j) -> p o j", o=1)
                        .to_broadcast((128, n_, Gseg)),
                    op=Alu.mult)
            if t0 + nt > THm:
                o0 = max(t0, THm)
                nb = t0 + nt - o0
                c0 = col_of(o0)
                nc.vector.tensor_tensor(
                    s["M"][:, c0:c0 + nb * 8]
                        .rearrange("p (t j) -> p t j", j=8),
                    s["P"][:, o0:o0 + nb].rearrange("p (t o) -> p t o", o=1)
                        .to_broadcast((128, nb, 8)),
                    Mk[0].rearrange("p (o j) -> p o j", o=1)
                        .to_broadcast((128, nb, 8)),
                    op=Alu.mult)

        def agg_chunk(p, ci):
            t0, nt = CH[p][ci]
            s = st[p]
            OV = OV_[p]
            x = xt[p][ci]
            for i in range(nt):
                tid = t0 + i
                c0 = col_of(tid)
                if tid < THm:
                    gt = tiles_tab[tid][1]
                    nc.tensor.matmul(
                        s["ynT"][:, c0:c0 + gt],
                        x[:, S * i:S * i + 128],
                        s["M"][:, c0:c0 + gt],
                        start=True, stop=True)
                else:
                    b = tid - THm
                    nc.tensor.matmul(
                        s["ov"][:, 8 * b:8 * b + 8],
                        x[:, S * i:S * i + 128],
                        s["M"][:, c0:c0 + 8],
                        start=True, stop=True)
            h0, h1 = t0, min(t0 + nt, THm)
            if h1 > h0:
                nc.tensor.matmul(
                    s["dn"][:, col_of(h0):col_of(h1 - 1) +
                            tiles_tab[h1 - 1][1]],
                    ones_col,
                    s["M"][:, col_of(h0):col_of(h1 - 1) +
                           tiles_tab[h1 - 1][1]],
                    start=True, stop=True)
            if t0 + nt > THm:
                o0 = max(t0, THm)
                nb = t0 + nt - o0
                nc.tensor.matmul(
                    s["ov"][0:1, 8 * OV:8 * OV + 8 * nb], ones_col,
                    s["M"][:, col_of(o0):col_of(o0) + 8 * nb],
                    start=True, stop=True)

        HS = GPC // 2

        def tail_ov(p):
            s = st[p]
            OV = OV_[p]
            ovsb = wk.tile([64, 16 * OV], f32, tag=f"ovsb{p}",
                           name=f"ovsb{p}")
            nc.scalar.activation(ovsb[:], s["ov"][0:64, 0:16 * OV],
                                 Act.Copy)
            nc.vector.tensor_tensor(
                s["ynT"][0:64, 0:8 * OV], s["ynT"][0:64, 0:8 * OV],
                ovsb[:, 0:8 * OV], op=Alu.add)
            nc.vector.tensor_tensor(
                s["dn"][:, 0:8 * OV], s["dn"][:, 0:8 * OV],
                ovsb[0:1, 8 * OV:16 * OV], op=Alu.add)

        def tail_a(p, h):
            s = st[p]
            c = slice(h * HS, (h + 1) * HS)
            nc.scalar.activation(s["dnb"][:, c], s["dn"][:, c], Act.Copy,
                                 bias=1e-16)
            nc.tensor.matmul(s["rbc"][:, c], ones64, s["dnb"][:, c],
                             start=True, stop=True)
            nc.vector.reciprocal_approx_fast(s["rinv"][:, c],
                                             s["rbc"][:, c])
            nc.vector.tensor_tensor(s["ynrm"][:, c], s["ynT"][0:64, c],
                                    s["rinv"][:, c], op=Alu.mult)

        def tail_b(p, h):
            s = st[p]
            c = slice(h * HS, (h + 1) * HS)
            nc.tensor.matmul(s["hT"][:, c], Wp[p], s["ynrm"][:, c],
                             start=True, stop=True)
            nc.scalar.activation(s["sg"][:, c], s["hT"][:, c], Act.Sigmoid,
                                 bias=pbias[p])

        # ---- schedule ----
        for ci in range(len(CH["u"])):
            for p in ("u", "d"):
                reduce_chunk(p, ci)
                mchain_chunk(p, ci)
                mbuild_chunk(p, ci)
                agg_chunk(p, ci)
        for p in ("u", "d"):
            tail_ov(p)
            for h in (1, 0):
                tail_a(p, h)
                tail_b(p, h)

        # ---- head ----
        prod = wk.tile([128, GPC], bf16, tag="prod")
        o_ps = pdn.tile([1, GPC], f32, tag="dn", name="o_ps")
        o_sb = wk.tile([1, GPC], f32, tag="o_sb")
        for h in (1, 0):
            c = slice(h * HS, (h + 1) * HS)
            nc.vector.tensor_tensor(prod[:, c], st["u"]["sg"][:, c],
                                    st["d"]["sg"][:, c], op=Alu.mult)
            nc.tensor.matmul(o_ps[:, c], mlpW, prod[:, c], start=True,
                             stop=True)
            nc.scalar.activation(o_sb[:, c], o_ps[:, c], Act.Copy)
        nc.sync.dma_start(out_dram.ap(), o_sb[:])

    nc.compile()
    return nc


def _get_module(regions, OVU, OVD):
    key = ("nc", regions, OVU, OVD)
    if key not in _CACHE:
        _CACHE[key] = _build_module(regions, OVU, OVD)
    return _CACHE[key]


# ---------------- host-side prep ----------------

def _branch_struct(ei):
    src = np.asarray(ei[0]).astype(np.int64)
    dst = np.asarray(ei[1]).astype(np.int64)
    valid = (dst % NPG) == (NPG - 1)
    cnt = np.bincount(src[valid], minlength=N).astype(np.float32)
    contrib = (cnt > 0).reshape(G, NPG).sum(1)
    return cnt, contrib


def _clamp_w(w):
    w = np.asarray(w, np.float64).copy()
    tiny = np.abs(w) < 1e-4
    w[tiny] = np.where(w[tiny] < 0, -1e-4, 1e-4)
    return w


def _plan_regions(orders, con_u, con_d):
    import os
    mode = os.environ.get("REGIONS", "1")
    if mode == "2":
        ngt8 = 0
        for order in orders:
            mx = np.maximum(con_u[order], con_d[order])
            ngt8 = max(ngt8, int((mx > 8).sum()))
        g16 = max(8, -(-ngt8 // 8) * 8)
        g8 = GPC - g16
        regions = ((16, 8, g16 // 8, g16), (8, 16, -(-g8 // 16), g8))
        assert regions[0][2] <= 32 and regions[1][2] <= 32
        return tuple(r for r in regions if r[2] > 0)
    return ((16, 8, GPC // 8, GPC),)


def _overflow_tiles(orders, cnt):
    nb = 0
    for order in orders:
        counts = np.array([(cnt[g * NPG:(g + 1) * NPG] > 0).sum()
                           for g in order])
        assert counts.max() <= 32, "needs level-2 overflow support"
        ranks = np.nonzero(counts > 16)[0]
        if len(ranks):
            nb = max(nb, int(ranks.max() // 8 + 1))
    return nb


def _pack_branch(x, cnt, orders, w_src, w_dst, regions, OV):
    import ml_dtypes
    bf = ml_dtypes.bfloat16
    x = np.asarray(x, np.float32)
    wc = _clamp_w(w_src).astype(np.float32)
    THm = sum(r[2] for r in regions)
    T = THm + OV
    rank0s = np.cumsum([0] + [r[3] for r in regions])
    t0s = np.cumsum([0] + [r[2] for r in regions])
    per_core = []
    for c in range(NC):
        order = orders[c]
        XN = np.zeros((128, (T + 1) * S), np.float32)
        CTm = np.zeros((128, T), np.float32)
        XLW = max(r[1] for r in regions) * S
        XLR = max(32 * i + r[2] for i, r in enumerate(regions))
        XL = np.zeros((XLR, XLW), np.float32)
        for r, g in enumerate(order):
            rid = int(np.searchsorted(rank0s, r, side="right") - 1)
            K, Gr, ntr, ga = regions[rid]
            rr = r - rank0s[rid]
            t, j = rr // Gr, rr % Gr
            tid = t0s[rid] + t
            xlrow = 32 * rid + t
            nodes = np.nonzero(cnt[g * NPG:(g + 1) * NPG] > 0)[0] + g * NPG
            assert len(nodes) <= K or rid == 0
            XL[xlrow, j * S:(j + 1) * S] = x[(g + 1) * NPG - 1] * w_dst
            seg = nodes[:K]
            p0 = K * j
            XN[p0:p0 + len(seg), tid * S:tid * S + S] = x[seg] * wc
            CTm[p0:p0 + len(seg), tid] = cnt[seg]
            if len(nodes) > K:          # overflow (region 0 only)
                assert rid == 0
                seg2 = nodes[K:]
                otid = THm + t
                p0 = 16 * j
                XN[p0:p0 + len(seg2), otid * S:otid * S + S] = x[seg2] * wc
                CTm[p0:p0 + len(seg2), otid] = cnt[seg2]
        per_core.append({"XN": XN.astype(bf), "CT": CTm,
                         "XL": XL.astype(bf)})
    return per_core, wc


def _build_in_maps(inputs):
    import ml_dtypes
    bf = ml_dtypes.bfloat16

    cnt_u, con_u = _branch_struct(inputs["up_edge_index"])
    cnt_d, con_d = _branch_struct(inputs["down_edge_index"])
    orders = []
    for c in range(NC):
        g0 = c * GPC
        mx = np.maximum(con_u[g0:g0 + GPC], con_d[g0:g0 + GPC])
        orders.append(np.argsort(-mx, kind="stable") + g0)
    regions = _plan_regions(orders, con_u, con_d)
    OVU = max(1, _overflow_tiles(orders, cnt_u))
    OVD = max(1, _overflow_tiles(orders, cnt_d))
    assert OVU <= regions[0][2] and OVD <= regions[0][2]
    THm = sum(r[2] for r in regions)
    TU, TD = THm + OVU, THm + OVD

    pcs = {}
    shr = {}
    for pref, p, cnt, OV in (("up", "u", cnt_u, OVU),
                             ("down", "d", cnt_d, OVD)):
        W = np.asarray(inputs[f"{pref}_W"], np.float32)
        w_src = W @ np.asarray(inputs[f"{pref}_att_src"], np.float32)
        w_dst = W @ np.asarray(inputs[f"{pref}_att_dst"], np.float32)
        pcs[p], wc = _pack_branch(inputs[f"{pref}_x"], cnt, orders,
                                  w_src, w_dst, regions, OV)
        shr[p] = {
            "Wp": (W / wc[:, None]).astype(np.float32),
            "bias": np.asarray(inputs[f"{pref}_bias"], np.float32),
        }

    FW = 100 + TU + TD
    cstF = np.zeros((128, FW), np.float32)
    cstF[:, 0] = shr["u"]["bias"]
    cstF[:, 1] = shr["d"]["bias"]
    cstF[0, 2] = 1e-16
    cstF[0:96, 4:100] = np.eye(96, dtype=np.float32)

    cstB = np.zeros((128, 768), np.float32)
    pp = np.arange(128)
    # region masks/Q: K=16 -> G=8 @ (0:128, 128:136); K=12 -> G=10;
    # K=8 -> G=16
    q16 = np.zeros((8, 128), np.float32)
    q16[pp // 16, pp] = 1.0
    cstB[0:8, 0:128] = q16
    cstB[:, 128:136] = q16.T
    q12 = np.zeros((10, 128), np.float32)
    sel = pp < 120
    q12[pp[sel] // 12, pp[sel]] = 1.0
    cstB[0:10, 458:586] = q12
    cstB[:, 714:724] = q12.T
    q8 = np.zeros((16, 128), np.float32)
    q8[pp // 8, pp] = 1.0
    cstB[0:16, 586:714] = q8
    cstB[:, 724:740] = q8.T
    cstB[:, 136] = 1.0
    cstB[0, 137:201] = 1.0
    cstB[0:64, 201:329] = shr["u"]["Wp"]
    cstB[0:64, 329:457] = shr["d"]["Wp"]
    cstB[:, 457] = np.asarray(inputs["mlp_W"], np.float32).reshape(H)

    in_maps = []
    for c in range(NC):
        cf = cstF.copy()
        cf[:, 100:100 + TU] = pcs["u"][c]["CT"]
        cf[:, 100 + TU:100 + TU + TD] = pcs["d"][c]["CT"]
        m = {
            "cstF": cf,
            "cstB": cstB.astype(bf),
            "u_xn": pcs["u"][c]["XN"],
            "d_xn": pcs["d"][c]["XN"],
            "u_xl": pcs["u"][c]["XL"],
            "d_xl": pcs["d"][c]["XL"],
        }
        in_maps.append(m)
    meta = {"orders": orders, "OVU": OVU, "OVD": OVD, "regions": regions,
            "mlp_b": float(np.asarray(inputs["mlp_b"]).reshape(-1)[0])}
    return in_maps, meta


def assemble(results, meta):
    out = np.zeros((G, 1), np.float32)
    for c in range(NC):
        o = np.asarray(results[c]["out"], np.float32).reshape(GPC)
        out[meta["orders"][c], 0] = o + meta["mlp_b"]
    return out


def kernel(**inputs):
    from concourse.bass_utils import run_bass_kernel_spmd

    in_maps, meta = _build_in_maps(inputs)
    nc = _get_module(meta["regions"], meta["OVU"], meta["OVD"])
    res = run_bass_kernel_spmd(nc, in_maps, core_ids=list(range(NC)))
    return assemble(res.results, meta)
